# revision 1
# baseline (speedup 1.0000x reference)
"""Multi-head attention (B=4, S=2048, D=1024, 16 heads x 64) on 8 trn2 cores.

Sharding: core c handles batch b = c//2 and head-group hg = c%2 (8 heads each,
i.e. columns hg*512:(hg+1)*512 of Wq/Wk/Wv and rows of Wo).  Each core returns
a partial output [S, D]; the host sums the two partials per batch and adds bo.

v5 (vs the serial baseline):
  * all matmul operands bf16 (inputs converted host-side): same PE rate as
    f32r, enables FWL weight loads, halves SBUF/DMA.
  * scores run as 64x128 row-tile PAIRS (two heads co-execute on rows 0:63 /
    64:127) in 4-sk mode-blocked bursts -- HW microbench: pairing is ~2x but
    only when tile modes aren't interleaved per-sk.
  * PV stays serial 65-wide (V plus ones column = softmax denominator), so no
    denominator bank is needed and the scores PSUM pool gets 3-deep
    double-buffering (6 banks) -- deep enough that ScalarE exp latency
    (~1.1us/unit) never stalls the scores burst.
  * exp: 2/3 of units exact on ScalarE, 1/3 on DVE via the Schraudolph
    bf16-bit trick (probs = bitcast(i16(s*A + B)), ~2% rms on those units;
    bias cancels in normalization).  Keeps ScalarE (the 218us-minimum engine)
    off the critical path.
  * phase-1/3 PSUM evictions on DVE so ScalarE does exp only.

build_nc(reps=N) wraps the body in a hardware For_i loop for the timing
harness (single-dispatch timing is impossible under the ~0.8ms axon RPC
floor); reps=1 emits the plain program used by kernel().
"""

import numpy as np
import ml_dtypes

import concourse.bass as bass
import concourse.tile as tile
from concourse import bacc, mybir
from concourse.bass_utils import run_bass_kernel_spmd

F32 = mybir.dt.float32
BF16 = mybir.dt.bfloat16
I16 = mybir.dt.int16
ACT = mybir.ActivationFunctionType
ALU = mybir.AluOpType

D = 1024          # d_model
HH = 512          # heads-per-core * head_dim = 8 * 64
HD = 64           # head dim
NHL = 8           # heads per core
B, S_FULL = 4, 2048
N_CORES = 8

LOG2E = 1.4426950408889634
SCHRAUD_A = 0.125 * LOG2E * 128.0          # scores scale 1/8 folded in
SCHRAUD_B = 127.0 * 128.0 - 5.5            # C=5.5: max rel 3.3%, rms 2.1%
EXP_DVE_EVERY = 0    # 0 = all exp exact on ScalarE (Schraudolph max-err is
                     # tail-dominated: peaked rows take the full ~3% hit)


def build_nc(S=S_FULL, reps=1):
    nc = bacc.Bacc("TRN2", target_bir_lowering=False, debug=False,
                   dynamic_dma_scratch_size=2048)

    xqT = nc.dram_tensor("xqT", [D, S], BF16, kind="ExternalInput").ap()
    xkT = nc.dram_tensor("xkT", [D, S], BF16, kind="ExternalInput").ap()
    xvT = nc.dram_tensor("xvT", [D, S], BF16, kind="ExternalInput").ap()
    wq = nc.dram_tensor("wq", [D, HH], BF16, kind="ExternalInput").ap()
    wk = nc.dram_tensor("wk", [D, HH], BF16, kind="ExternalInput").ap()
    wv = nc.dram_tensor("wv", [D, HH], BF16, kind="ExternalInput").ap()
    wo = nc.dram_tensor("wo", [HH, D], BF16, kind="ExternalInput").ap()
    bq = nc.dram_tensor("bq", [HH], F32, kind="ExternalInput").ap()
    bk = nc.dram_tensor("bk", [HH], F32, kind="ExternalInput").ap()
    bv = nc.dram_tensor("bv", [HH], F32, kind="ExternalInput").ap()
    out = nc.dram_tensor("out", [S, D], F32, kind="ExternalOutput").ap()

    NT = S // 512        # 512-token chunks
    NSK = S // 128       # 128-token key tiles
    NKT = D // 128       # 128-wide d_model tiles
    NKB = HH // 128      # 128-wide hidden tiles (head pairs)
    DD = HD + 1          # V head-group width (64 values + ones column)

    with tile.TileContext(nc) as tc:
        from contextlib import ExitStack

        rep_loop = tc.For_i(0, reps, 1) if reps > 1 else None
        if rep_loop is not None:
            rep_loop.__enter__()

        with ExitStack() as ctx:
            persist = ctx.enter_context(tc.tile_pool(name="persist", bufs=1))
            qt_sb = persist.tile([128, NKB, S], BF16, tag="qt")
            kt_sb = persist.tile([128, NKB, S], BF16, tag="kt")
            vb_sb = persist.tile([128, NSK, NHL * DD], BF16, tag="vb")
            zt_sb = persist.tile([128, NKB, S], BF16, tag="zt")
            wo_sb = persist.tile([128, NKB, D], BF16, tag="wo")
            bq_sb = persist.tile([128, NKB], F32, tag="bq")
            bk_sb = persist.tile([128, NKB], F32, tag="bk")
            bvb_sb = persist.tile([128, HH], F32, tag="bvb")

            nc.sync.dma_start(out=bq_sb, in_=bq.rearrange("(kb p) -> p kb", p=128))
            nc.sync.dma_start(out=bk_sb, in_=bk.rearrange("(kb p) -> p kb", p=128))
            bv_bcast_in = bass.AP(tensor=bv.tensor, offset=bv.offset,
                                  ap=[[0, 128], [1, HH]])
            nc.sync.dma_start(out=bvb_sb, in_=bv_bcast_in)
            # ones columns of V~ (softmax denominator trick)
            ones_view = vb_sb.rearrange("p s (h dd) -> p s h dd", dd=DD)[:, :, :, HD:HD + 1]
            nc.vector.memset(ones_view, 1.0)
            # preload the exp ACT table before the pipeline needs it
            warm = persist.tile([1, 1], BF16, tag="warm")
            nc.scalar.activation(warm, bq_sb[0:1, 0:1], ACT.Exp, scale=1.0)

            # ---------------- phase 1: projections ----------------
            with ExitStack() as c1:
                wpool = c1.enter_context(tc.tile_pool(name="wpool", bufs=2))
                xpool = c1.enter_context(tc.tile_pool(name="xpool", bufs=3))
                p1 = c1.enter_context(tc.tile_pool(name="p1", bufs=4, space="PSUM"))

                # K first: scores for chunk c need full KT but only chunk c of QT
                for (xT, w_dram, dst, bias) in ((xkT, wk, kt_sb, bk_sb),
                                                (xqT, wq, qt_sb, bq_sb)):
                    w_sb = wpool.tile([128, NKT, HH], BF16, tag="w",
                                      name=f"w_{dst.name}")
                    nc.sync.dma_start(out=w_sb,
                                      in_=w_dram.rearrange("(kt p) n -> p kt n", p=128))
                    for t in range(NT):
                        xt = xpool.tile([128, NKT, 512], BF16, tag="xt")
                        nc.sync.dma_start(
                            out=xt,
                            in_=xT.rearrange("(kt p) s -> p kt s", p=128)[:, :, t * 512:(t + 1) * 512])
                        for kb in range(NKB):
                            ps = p1.tile([128, 512], F32, tag="ps1")
                            for kt in range(NKT):
                                nc.tensor.matmul(
                                    ps,
                                    lhsT=w_sb[:, kt, kb * 128:(kb + 1) * 128],
                                    rhs=xt[:, kt, :],
                                    start=(kt == 0), stop=(kt == NKT - 1))
                            nc.vector.tensor_scalar_add(
                                dst[:, kb, t * 512:(t + 1) * 512], ps,
                                bias[:, kb:kb + 1])

                # V pass: natural [tokens, hidden] with 65-wide head groups
                wv_sb = wpool.tile([128, NKT, HH], BF16, tag="w", name="w_v")
                nc.sync.dma_start(out=wv_sb,
                                  in_=wv.rearrange("(kt p) n -> p kt n", p=128))
                for t in range(NT):
                    xt = xpool.tile([128, NKT, 512], BF16, tag="xt")
                    nc.sync.dma_start(
                        out=xt,
                        in_=xvT.rearrange("(kt p) s -> p kt s", p=128)[:, :, t * 512:(t + 1) * 512])
                    for m in range(4):
                        ps = p1.tile([128, 512], F32, tag="ps1")
                        for kt in range(NKT):
                            nc.tensor.matmul(
                                ps,
                                lhsT=xt[:, kt, m * 128:(m + 1) * 128],
                                rhs=wv_sb[:, kt, :],
                                start=(kt == 0), stop=(kt == NKT - 1))
                        sk = t * 4 + m
                        vdst = vb_sb[:, sk, :].rearrange(
                            "p (h dd) -> p h dd", dd=DD)[:, :, 0:HD]
                        nc.vector.tensor_add(
                            vdst,
                            ps.rearrange("p (h d) -> p h d", d=HD),
                            bvb_sb.rearrange("p (h d) -> p h d", d=HD))

            # ---------------- phase 2: attention ----------------
            exp_unit = 0
            with ExitStack() as c2:
                sppool = c2.enter_context(tc.tile_pool(name="sppool", bufs=3, space="PSUM"))
                zqpool = c2.enter_context(tc.tile_pool(name="zqpool", bufs=1, space="PSUM"))
                ptpool = c2.enter_context(tc.tile_pool(name="ptpool", bufs=10))
                npool = c2.enter_context(tc.tile_pool(name="npool", bufs=2))

                BLK = 4
                NB = NSK // BLK
                for c in range(NT):
                    for kb in range(NKB):
                        zps = [zqpool.tile([DD, 512], F32, tag=f"z{d}",
                                           name=f"zps{d}_{kb}_{c}")
                               for d in range(2)]
                        pts = {}
                        for blk in range(NB + 1):
                            if blk < NB:
                                # scores burst: 64x128 row-pair mode
                                for sk in range(blk * BLK, (blk + 1) * BLK):
                                    spt = sppool.tile([128, 2, 512], F32, tag="sp")
                                    for d in range(2):
                                        nc.tensor.matmul(
                                            spt[:, d, :],
                                            lhsT=kt_sb[d * 64:(d + 1) * 64, kb,
                                                       sk * 128:(sk + 1) * 128],
                                            rhs=qt_sb[d * 64:(d + 1) * 64, kb,
                                                      c * 512:(c + 1) * 512],
                                            start=True, stop=True)
                                    pt = ptpool.tile([128, 2, 512], BF16, tag="pt")
                                    if EXP_DVE_EVERY and \
                                            exp_unit % EXP_DVE_EVERY == EXP_DVE_EVERY - 1:
                                        nc.vector.tensor_scalar(
                                            pt.bitcast(I16), spt,
                                            SCHRAUD_A, SCHRAUD_B,
                                            ALU.mult, ALU.add)
                                    else:
                                        nc.scalar.activation(pt, spt, ACT.Exp,
                                                             scale=0.125)
                                    exp_unit += 1
                                    pts[sk] = pt
                            if blk >= 1:
                                # PV burst for previous block: 65-wide serial
                                for j in range((blk - 1) * BLK, blk * BLK):
                                    pt = pts.pop(j)
                                    for d in range(2):
                                        hh = 2 * kb + d
                                        nc.tensor.matmul(
                                            zps[d],
                                            lhsT=vb_sb[:, j, hh * DD:(hh + 1) * DD],
                                            rhs=pt[:, d, :],
                                            start=(j == 0), stop=(j == NSK - 1))

                        # normalize: z / denominator-row -> zt bf16
                        for d in range(2):
                            dcp = npool.tile([1, 512], F32, tag="dcp")
                            nc.vector.tensor_copy(dcp, zps[d][HD:HD + 1, :])
                            rc = npool.tile([1, 512], F32, tag="rc")
                            nc.vector.reciprocal_approx_fast(rc, dcp)
                            bc = npool.tile([HD, 512], F32, tag="bc")
                            nc.gpsimd.partition_broadcast(bc, rc, channels=HD)
                            nc.vector.tensor_mul(
                                zt_sb[d * 64:(d + 1) * 64, kb, c * 512:(c + 1) * 512],
                                zps[d][0:HD, :], bc)

            # ---------------- phase 3: output projection ----------------
            nc.sync.dma_start(out=wo_sb, in_=wo.rearrange("(hb p) n -> p hb n", p=128))
            with ExitStack() as c3:
                opool = c3.enter_context(tc.tile_pool(name="opool", bufs=3))
                p3 = c3.enter_context(tc.tile_pool(name="p3", bufs=3, space="PSUM"))
                for t in range(S // 128):
                    os_t = opool.tile([128, D], F32, tag="os")
                    for n in range(D // 512):
                        po = p3.tile([128, 512], F32, tag="po")
                        for hb in range(NKB):
                            nc.tensor.matmul(
                                po,
                                lhsT=zt_sb[:, hb, t * 128:(t + 1) * 128],
                                rhs=wo_sb[:, hb, n * 512:(n + 1) * 512],
                                start=(hb == 0), stop=(hb == NKB - 1))
                        nc.vector.tensor_copy(os_t[:, n * 512:(n + 1) * 512], po)
                    nc.sync.dma_start(out=out[t * 128:(t + 1) * 128, :], in_=os_t)

        if rep_loop is not None:
            rep_loop.__exit__(None, None, None)

    nc.compile()
    return nc


_NC_CACHE = {}


def _get_nc(S=S_FULL, reps=1):
    key = (S, reps)
    if key not in _NC_CACHE:
        _NC_CACHE[key] = build_nc(S, reps=reps)
    return _NC_CACHE[key]


def make_in_maps(query, key, value, Wq, bq, Wk, bk, Wv, bv, Wo, bo):
    """Shard full inputs into 8 per-core input dicts (bf16 operands)."""
    bf = lambda a: np.ascontiguousarray(np.asarray(a, dtype=np.float32)).astype(ml_dtypes.bfloat16)
    f32 = lambda a: np.ascontiguousarray(np.asarray(a, dtype=np.float32))
    in_maps = []
    for core in range(N_CORES):
        b, hg = core // 2, core % 2
        sl = slice(hg * HH, (hg + 1) * HH)
        in_maps.append({
            "xqT": bf(np.asarray(query)[b].T),
            "xkT": bf(np.asarray(key)[b].T),
            "xvT": bf(np.asarray(value)[b].T),
            "wq": bf(np.asarray(Wq)[:, sl]),
            "wk": bf(np.asarray(Wk)[:, sl]),
            "wv": bf(np.asarray(Wv)[:, sl]),
            "wo": bf(np.asarray(Wo)[sl, :]),
            "bq": f32(np.asarray(bq)[sl]),
            "bk": f32(np.asarray(bk)[sl]),
            "bv": f32(np.asarray(bv)[sl]),
        })
    return in_maps


def kernel(query, key, value, Wq, bq, Wk, bk, Wv, bv, Wo, bo, **run_kwargs):
    nc = _get_nc(S_FULL)
    in_maps = make_in_maps(query, key, value, Wq, bq, Wk, bk, Wv, bv, Wo, bo)
    res = run_bass_kernel_spmd(nc, in_maps, core_ids=list(range(N_CORES)),
                               **run_kwargs)
    bo_np = np.asarray(bo, dtype=np.float32)
    outs = [np.asarray(r["out"], dtype=np.float32) for r in res.results]
    full = np.stack([outs[2 * b] + outs[2 * b + 1] + bo_np for b in range(B)])
    return full.astype(np.float32)



# revision 2
# speedup vs baseline: 1.0117x; 1.0117x over previous
"""Multi-head attention (B=4, S=2048, D=1024, 16 heads x 64) on 8 trn2 cores.

Sharding: core c handles batch b = c//2 and head-group hg = c%2 (8 heads each,
i.e. columns hg*512:(hg+1)*512 of Wq/Wk/Wv and rows of Wo).  Each core returns
a partial output [S, D]; the host sums the two partials per batch and adds bo.

v6 (vs v5's phase-serial structure):
  * single flat pool scope -- no phase ExitStacks, so the Tile scheduler can
    overlap Q-projection of chunk c+1 and out-projection of chunk c-1 with the
    exp-bound attention of chunk c (HW: ScalarE exp is (N+352)/1.2ns = 294us
    for all 33.5M probs; PE total is ~318us; v5 serialized them).
  * exp split across engines: 2/3 of score tiles take exact Exp on ScalarE
    (1147ns/tile measured), 1/3 take the Schraudolph bf16-bit trick on DVE
    (1281ns/tile from PSUM).  Bias mostly cancels in the softmax
    normalization; error checked against the 2e-2 gate.
  * emission order = scheduler priority: K-proj, Q-proj(0), V-proj, then per
    chunk [attention(c), Q-proj(c+1), out-proj(c)].
  * PSUM: scores 2x[128,2,512] (4 banks) + zq 2x[65,512] (2) + shared
    proj/outproj pool 2x[128,512] (2) = 8 banks.

build_nc(reps=N) wraps the body in a hardware For_i loop for the timing
harness; reps=1 emits the plain program used by kernel().
"""

import numpy as np
import ml_dtypes

import concourse.bass as bass
import concourse.tile as tile
from concourse import bacc, mybir
from concourse.bass_utils import run_bass_kernel_spmd

F32 = mybir.dt.float32
BF16 = mybir.dt.bfloat16
I16 = mybir.dt.int16
ACT = mybir.ActivationFunctionType
ALU = mybir.AluOpType

D = 1024          # d_model
HH = 512          # heads-per-core * head_dim = 8 * 64
HD = 64           # head dim
NHL = 8           # heads per core
B, S_FULL = 4, 2048
N_CORES = 8

LOG2E = 1.4426950408889634
SCHRAUD_A = 0.125 * LOG2E * 128.0          # scores scale 1/8 folded in
SCHRAUD_B = 127.0 * 128.0 - 5.5            # C=5.5: max rel 3.3%, rms 2.1%
EXP_DVE_EVERY = 3    # every 3rd score tile exps on DVE via Schraudolph


def build_nc(S=S_FULL, reps=1):
    nc = bacc.Bacc("TRN2", target_bir_lowering=False, debug=False,
                   dynamic_dma_scratch_size=2048)

    xqT = nc.dram_tensor("xqT", [D, S], BF16, kind="ExternalInput").ap()
    xkT = nc.dram_tensor("xkT", [D, S], BF16, kind="ExternalInput").ap()
    xvT = nc.dram_tensor("xvT", [D, S], BF16, kind="ExternalInput").ap()
    wq = nc.dram_tensor("wq", [D, HH], BF16, kind="ExternalInput").ap()
    wk = nc.dram_tensor("wk", [D, HH], BF16, kind="ExternalInput").ap()
    wv = nc.dram_tensor("wv", [D, HH], BF16, kind="ExternalInput").ap()
    wo = nc.dram_tensor("wo", [HH, D], BF16, kind="ExternalInput").ap()
    bq = nc.dram_tensor("bq", [HH], F32, kind="ExternalInput").ap()
    bk = nc.dram_tensor("bk", [HH], F32, kind="ExternalInput").ap()
    bv = nc.dram_tensor("bv", [HH], F32, kind="ExternalInput").ap()
    out = nc.dram_tensor("out", [S, D], F32, kind="ExternalOutput").ap()

    NT = S // 512        # 512-token chunks
    NSK = S // 128       # 128-token key tiles
    NKT = D // 128       # 128-wide d_model tiles
    NKB = HH // 128      # 128-wide hidden tiles (head pairs)
    DD = HD + 1          # V head-group width (64 values + ones column)

    with tile.TileContext(nc) as tc:
        from contextlib import ExitStack

        rep_loop = tc.For_i(0, reps, 1) if reps > 1 else None
        if rep_loop is not None:
            rep_loop.__enter__()

        with ExitStack() as ctx:
            persist = ctx.enter_context(tc.tile_pool(name="persist", bufs=1))
            qt_sb = persist.tile([128, NKB, S], BF16, tag="qt")
            kt_sb = persist.tile([128, NKB, S], BF16, tag="kt")
            vb_sb = persist.tile([128, NSK, NHL * DD], BF16, tag="vb")
            zt_sb = persist.tile([128, NKB, S], BF16, tag="zt")
            wo_sb = persist.tile([128, NKB, D], BF16, tag="wo")
            wq_sb = persist.tile([128, NKT, HH], BF16, tag="wqs")
            wk_sb = persist.tile([128, NKT, HH], BF16, tag="wks")
            wv_sb = persist.tile([128, NKT, HH], BF16, tag="wvs")
            bq_sb = persist.tile([128, NKB], F32, tag="bq")
            bk_sb = persist.tile([128, NKB], F32, tag="bk")
            bvb_sb = persist.tile([128, HH], F32, tag="bvb")

            xpool = ctx.enter_context(tc.tile_pool(name="xpool", bufs=3))
            ptpool = ctx.enter_context(tc.tile_pool(name="ptpool", bufs=10))
            npool = ctx.enter_context(tc.tile_pool(name="npool", bufs=4))
            opool = ctx.enter_context(tc.tile_pool(name="opool", bufs=2))
            pp = ctx.enter_context(tc.tile_pool(name="pp", bufs=2, space="PSUM"))
            sppool = ctx.enter_context(tc.tile_pool(name="sp", bufs=2, space="PSUM"))
            zqpool = ctx.enter_context(tc.tile_pool(name="zq", bufs=1, space="PSUM"))

            nc.sync.dma_start(out=bq_sb, in_=bq.rearrange("(kb p) -> p kb", p=128))
            nc.sync.dma_start(out=bk_sb, in_=bk.rearrange("(kb p) -> p kb", p=128))
            bv_bcast_in = bass.AP(tensor=bv.tensor, offset=bv.offset,
                                  ap=[[0, 128], [1, HH]])
            nc.sync.dma_start(out=bvb_sb, in_=bv_bcast_in)
            nc.sync.dma_start(out=wk_sb,
                              in_=wk.rearrange("(kt p) n -> p kt n", p=128))
            nc.sync.dma_start(out=wq_sb,
                              in_=wq.rearrange("(kt p) n -> p kt n", p=128))
            nc.sync.dma_start(out=wv_sb,
                              in_=wv.rearrange("(kt p) n -> p kt n", p=128))
            nc.sync.dma_start(out=wo_sb, in_=wo.rearrange("(hb p) n -> p hb n", p=128))
            # ones columns of V~ (softmax denominator trick)
            ones_view = vb_sb.rearrange("p s (h dd) -> p s h dd", dd=DD)[:, :, :, HD:HD + 1]
            nc.vector.memset(ones_view, 1.0)
            # preload the exp ACT table before the pipeline needs it
            warm = persist.tile([1, 1], BF16, tag="warm")
            nc.scalar.activation(warm, bq_sb[0:1, 0:1], ACT.Exp, scale=1.0)

            def proj_qk(xT, w_sb, dst, bias, t):
                xt = xpool.tile([128, NKT, 512], BF16, tag="xt",
                                name=f"xt_{dst.name}_{t}")
                nc.sync.dma_start(
                    out=xt,
                    in_=xT.rearrange("(kt p) s -> p kt s", p=128)[:, :, t * 512:(t + 1) * 512])
                for kb in range(NKB):
                    ps = pp.tile([128, 512], F32, tag="pp",
                                 name=f"ps_{dst.name}_{t}_{kb}")
                    for kt in range(NKT):
                        nc.tensor.matmul(
                            ps,
                            lhsT=w_sb[:, kt, kb * 128:(kb + 1) * 128],
                            rhs=xt[:, kt, :],
                            start=(kt == 0), stop=(kt == NKT - 1))
                    nc.vector.tensor_scalar_add(
                        dst[:, kb, t * 512:(t + 1) * 512], ps,
                        bias[:, kb:kb + 1])

            def proj_v(t):
                xt = xpool.tile([128, NKT, 512], BF16, tag="xt",
                                name=f"xt_v_{t}")
                nc.sync.dma_start(
                    out=xt,
                    in_=xvT.rearrange("(kt p) s -> p kt s", p=128)[:, :, t * 512:(t + 1) * 512])
                for m in range(4):
                    ps = pp.tile([128, 512], F32, tag="pp", name=f"ps_v_{t}_{m}")
                    for kt in range(NKT):
                        nc.tensor.matmul(
                            ps,
                            lhsT=xt[:, kt, m * 128:(m + 1) * 128],
                            rhs=wv_sb[:, kt, :],
                            start=(kt == 0), stop=(kt == NKT - 1))
                    sk = t * 4 + m
                    vdst = vb_sb[:, sk, :].rearrange(
                        "p (h dd) -> p h dd", dd=DD)[:, :, 0:HD]
                    nc.vector.tensor_add(
                        vdst,
                        ps.rearrange("p (h d) -> p h d", d=HD),
                        bvb_sb.rearrange("p (h d) -> p h d", d=HD))

            exp_state = [0]

            def attn(c):
                BLK = 4
                NB = NSK // BLK
                for kb in range(NKB):
                    zps = [zqpool.tile([DD, 512], F32, tag=f"z{d}",
                                       name=f"zps{d}_{kb}_{c}")
                           for d in range(2)]
                    pts = {}
                    for blk in range(NB + 1):
                        if blk < NB:
                            # scores burst: 64x128 row-pair mode
                            for sk in range(blk * BLK, (blk + 1) * BLK):
                                spt = sppool.tile([128, 2, 512], F32, tag="sp")
                                for d in range(2):
                                    nc.tensor.matmul(
                                        spt[:, d, :],
                                        lhsT=kt_sb[d * 64:(d + 1) * 64, kb,
                                                   sk * 128:(sk + 1) * 128],
                                        rhs=qt_sb[d * 64:(d + 1) * 64, kb,
                                                  c * 512:(c + 1) * 512],
                                        start=True, stop=True)
                                pt = ptpool.tile([128, 2, 512], BF16, tag="pt")
                                u = exp_state[0]
                                if EXP_DVE_EVERY and \
                                        u % EXP_DVE_EVERY == EXP_DVE_EVERY - 1:
                                    nc.vector.tensor_scalar(
                                        pt.bitcast(I16), spt,
                                        SCHRAUD_A, SCHRAUD_B,
                                        ALU.mult, ALU.add)
                                else:
                                    nc.scalar.activation(pt, spt, ACT.Exp,
                                                         scale=0.125)
                                exp_state[0] = u + 1
                                pts[sk] = pt
                        if blk >= 1:
                            # PV burst for previous block: 65-wide serial
                            for j in range((blk - 1) * BLK, blk * BLK):
                                pt = pts.pop(j)
                                for d in range(2):
                                    hh = 2 * kb + d
                                    nc.tensor.matmul(
                                        zps[d],
                                        lhsT=vb_sb[:, j, hh * DD:(hh + 1) * DD],
                                        rhs=pt[:, d, :],
                                        start=(j == 0), stop=(j == NSK - 1))

                    # normalize: z / denominator-row -> zt bf16
                    for d in range(2):
                        rc = npool.tile([1, 512], F32, tag="rc",
                                        name=f"rc_{c}_{kb}_{d}")
                        nc.vector.reciprocal_approx_fast(rc, zps[d][HD:HD + 1, :])
                        bc = npool.tile([HD, 512], F32, tag="bc",
                                        name=f"bc_{c}_{kb}_{d}")
                        nc.gpsimd.partition_broadcast(bc, rc, channels=HD)
                        nc.vector.tensor_mul(
                            zt_sb[d * 64:(d + 1) * 64, kb, c * 512:(c + 1) * 512],
                            zps[d][0:HD, :], bc)

            def outproj(c):
                for t in range(4 * c, 4 * c + 4):
                    os_t = opool.tile([128, D], F32, tag="os", name=f"os_{t}")
                    for n in range(D // 512):
                        po = pp.tile([128, 512], F32, tag="pp",
                                     name=f"po_{t}_{n}")
                        for hb in range(NKB):
                            nc.tensor.matmul(
                                po,
                                lhsT=zt_sb[:, hb, t * 128:(t + 1) * 128],
                                rhs=wo_sb[:, hb, n * 512:(n + 1) * 512],
                                start=(hb == 0), stop=(hb == NKB - 1))
                        nc.vector.tensor_copy(os_t[:, n * 512:(n + 1) * 512], po)
                    nc.sync.dma_start(out=out[t * 128:(t + 1) * 128, :], in_=os_t)

            # ---------------- emission (= scheduler priority) ----------------
            for t in range(NT):
                proj_qk(xkT, wk_sb, kt_sb, bk_sb, t)
            proj_qk(xqT, wq_sb, qt_sb, bq_sb, 0)
            for t in range(NT):
                proj_v(t)
            for c in range(NT):
                attn(c)
                if c + 1 < NT:
                    proj_qk(xqT, wq_sb, qt_sb, bq_sb, c + 1)
                outproj(c)

        if rep_loop is not None:
            rep_loop.__exit__(None, None, None)

    nc.compile()
    return nc


_NC_CACHE = {}


def _get_nc(S=S_FULL, reps=1):
    key = (S, reps)
    if key not in _NC_CACHE:
        _NC_CACHE[key] = build_nc(S, reps=reps)
    return _NC_CACHE[key]


def make_in_maps(query, key, value, Wq, bq, Wk, bk, Wv, bv, Wo, bo):
    """Shard full inputs into 8 per-core input dicts (bf16 operands)."""
    bf = lambda a: np.ascontiguousarray(np.asarray(a, dtype=np.float32)).astype(ml_dtypes.bfloat16)
    f32 = lambda a: np.ascontiguousarray(np.asarray(a, dtype=np.float32))
    in_maps = []
    for core in range(N_CORES):
        b, hg = core // 2, core % 2
        sl = slice(hg * HH, (hg + 1) * HH)
        in_maps.append({
            "xqT": bf(np.asarray(query)[b].T),
            "xkT": bf(np.asarray(key)[b].T),
            "xvT": bf(np.asarray(value)[b].T),
            "wq": bf(np.asarray(Wq)[:, sl]),
            "wk": bf(np.asarray(Wk)[:, sl]),
            "wv": bf(np.asarray(Wv)[:, sl]),
            "wo": bf(np.asarray(Wo)[sl, :]),
            "bq": f32(np.asarray(bq)[sl]),
            "bk": f32(np.asarray(bk)[sl]),
            "bv": f32(np.asarray(bv)[sl]),
        })
    return in_maps


def kernel(query, key, value, Wq, bq, Wk, bk, Wv, bv, Wo, bo, **run_kwargs):
    nc = _get_nc(S_FULL)
    in_maps = make_in_maps(query, key, value, Wq, bq, Wk, bk, Wv, bv, Wo, bo)
    res = run_bass_kernel_spmd(nc, in_maps, core_ids=list(range(N_CORES)),
                               **run_kwargs)
    bo_np = np.asarray(bo, dtype=np.float32)
    outs = [np.asarray(r["out"], dtype=np.float32) for r in res.results]
    full = np.stack([outs[2 * b] + outs[2 * b + 1] + bo_np for b in range(B)])
    return full.astype(np.float32)


# revision 3
# speedup vs baseline: 1.0520x; 1.0399x over previous
"""Multi-head attention (B=4, S=2048, D=1024, 16 heads x 64) on 8 trn2 cores.

Sharding: core c handles batch b = c//2 and head-group hg = c%2 (8 heads each,
i.e. columns hg*512:(hg+1)*512 of Wq/Wk/Wv and rows of Wo).  Each core returns
a partial output [S, D]; the host sums the two partials per batch and adds bo.

v6 (vs v5's phase-serial structure):
  * single flat pool scope -- no phase ExitStacks, so the Tile scheduler can
    overlap Q-projection of chunk c+1 and out-projection of chunk c-1 with the
    exp-bound attention of chunk c (HW: ScalarE exp is (N+352)/1.2ns = 294us
    for all 33.5M probs; PE total is ~318us; v5 serialized them).
  * exp split across engines: 2/3 of score tiles take exact Exp on ScalarE
    (1147ns/tile measured), 1/3 take the Schraudolph bf16-bit trick on DVE
    (1281ns/tile from PSUM).  Bias mostly cancels in the softmax
    normalization; error checked against the 2e-2 gate.
  * emission order = scheduler priority: K-proj, Q-proj(0), V-proj, then per
    chunk [attention(c), Q-proj(c+1), out-proj(c)].
  * PSUM: scores 2x[128,2,512] (4 banks) + zq 2x[65,512] (2) + shared
    proj/outproj pool 2x[128,512] (2) = 8 banks.

build_nc(reps=N) wraps the body in a hardware For_i loop for the timing
harness; reps=1 emits the plain program used by kernel().
"""

import numpy as np
import ml_dtypes

import concourse.bass as bass
import concourse.tile as tile
from concourse import bacc, mybir
from concourse.bass_utils import run_bass_kernel_spmd

F32 = mybir.dt.float32
BF16 = mybir.dt.bfloat16
I16 = mybir.dt.int16
ACT = mybir.ActivationFunctionType
ALU = mybir.AluOpType

D = 1024          # d_model
HH = 512          # heads-per-core * head_dim = 8 * 64
HD = 64           # head dim
NHL = 8           # heads per core
B, S_FULL = 4, 2048
N_CORES = 8

LOG2E = 1.4426950408889634
SCHRAUD_A = 0.125 * LOG2E * 128.0          # scores scale 1/8 folded in
SCHRAUD_B = 127.0 * 128.0 - 5.5            # C=5.5: max rel 3.3%, rms 2.1%
EXP_DVE_EVERY = 3    # every 3rd score tile exps on DVE via Schraudolph


def build_nc(S=S_FULL, reps=1):
    nc = bacc.Bacc("TRN2", target_bir_lowering=False, debug=False,
                   dynamic_dma_scratch_size=2048)

    xqT = nc.dram_tensor("xqT", [D, S], BF16, kind="ExternalInput").ap()
    xkT = nc.dram_tensor("xkT", [D, S], BF16, kind="ExternalInput").ap()
    xvT = nc.dram_tensor("xvT", [D, S], BF16, kind="ExternalInput").ap()
    wq = nc.dram_tensor("wq", [D, HH], BF16, kind="ExternalInput").ap()
    wk = nc.dram_tensor("wk", [D, HH], BF16, kind="ExternalInput").ap()
    wv = nc.dram_tensor("wv", [D, HH], BF16, kind="ExternalInput").ap()
    wo = nc.dram_tensor("wo", [HH, D], BF16, kind="ExternalInput").ap()
    bq = nc.dram_tensor("bq", [HH], F32, kind="ExternalInput").ap()
    bk = nc.dram_tensor("bk", [HH], F32, kind="ExternalInput").ap()
    bv = nc.dram_tensor("bv", [HH], F32, kind="ExternalInput").ap()
    out = nc.dram_tensor("out", [S, D], F32, kind="ExternalOutput").ap()

    NT = S // 512        # 512-token chunks
    NSK = S // 128       # 128-token key tiles
    NKT = D // 128       # 128-wide d_model tiles
    NKB = HH // 128      # 128-wide hidden tiles (head pairs)
    DD = HD + 1          # V head-group width (64 values + ones column)

    with tile.TileContext(nc) as tc:
        from contextlib import ExitStack

        rep_loop = tc.For_i(0, reps, 1) if reps > 1 else None
        if rep_loop is not None:
            rep_loop.__enter__()

        with ExitStack() as ctx:
            persist = ctx.enter_context(tc.tile_pool(name="persist", bufs=1))
            qt_sb = persist.tile([128, NKB, S], BF16, tag="qt")
            kt_sb = persist.tile([128, NKB, S], BF16, tag="kt")
            vb_sb = persist.tile([128, NSK, NHL * DD], BF16, tag="vb")
            zt_sb = persist.tile([128, NKB, S], BF16, tag="zt")
            wo_sb = persist.tile([128, NKB, D], BF16, tag="wo")
            wq_sb = persist.tile([128, NKT, HH], BF16, tag="wqs")
            wk_sb = persist.tile([128, NKT, HH], BF16, tag="wks")
            wv_sb = persist.tile([128, NKT, HH], BF16, tag="wvs")
            bq_sb = persist.tile([128, NKB], F32, tag="bq")
            bk_sb = persist.tile([128, NKB], F32, tag="bk")
            bvb_sb = persist.tile([128, HH], F32, tag="bvb")

            xpool = ctx.enter_context(tc.tile_pool(name="xpool", bufs=3))
            ptpool = ctx.enter_context(tc.tile_pool(name="ptpool", bufs=10))
            npool = ctx.enter_context(tc.tile_pool(name="npool", bufs=4))
            opool = ctx.enter_context(tc.tile_pool(name="opool", bufs=2))
            pp = ctx.enter_context(tc.tile_pool(name="pp", bufs=2, space="PSUM"))
            sppool = ctx.enter_context(tc.tile_pool(name="sp", bufs=2, space="PSUM"))
            zqpool = ctx.enter_context(tc.tile_pool(name="zq", bufs=1, space="PSUM"))

            nc.sync.dma_start(out=bq_sb, in_=bq.rearrange("(kb p) -> p kb", p=128))
            nc.sync.dma_start(out=bk_sb, in_=bk.rearrange("(kb p) -> p kb", p=128))
            bv_bcast_in = bass.AP(tensor=bv.tensor, offset=bv.offset,
                                  ap=[[0, 128], [1, HH]])
            nc.sync.dma_start(out=bvb_sb, in_=bv_bcast_in)
            nc.sync.dma_start(out=wk_sb,
                              in_=wk.rearrange("(kt p) n -> p kt n", p=128))
            nc.sync.dma_start(out=wq_sb,
                              in_=wq.rearrange("(kt p) n -> p kt n", p=128))
            nc.sync.dma_start(out=wv_sb,
                              in_=wv.rearrange("(kt p) n -> p kt n", p=128))
            nc.sync.dma_start(out=wo_sb, in_=wo.rearrange("(hb p) n -> p hb n", p=128))
            # ones columns of V~ (softmax denominator trick)
            ones_view = vb_sb.rearrange("p s (h dd) -> p s h dd", dd=DD)[:, :, :, HD:HD + 1]
            nc.vector.memset(ones_view, 1.0)
            # preload the exp ACT table before the pipeline needs it
            warm = persist.tile([1, 1], BF16, tag="warm")
            nc.scalar.activation(warm, bq_sb[0:1, 0:1], ACT.Exp, scale=1.0)

            def proj_qk(xT, w_sb, dst, bias, t):
                xt = xpool.tile([128, NKT, 512], BF16, tag="xt",
                                name=f"xt_{dst.name}_{t}")
                nc.sync.dma_start(
                    out=xt,
                    in_=xT.rearrange("(kt p) s -> p kt s", p=128)[:, :, t * 512:(t + 1) * 512])
                for kb in range(NKB):
                    ps = pp.tile([128, 512], F32, tag="pp",
                                 name=f"ps_{dst.name}_{t}_{kb}")
                    for kt in range(NKT):
                        nc.tensor.matmul(
                            ps,
                            lhsT=w_sb[:, kt, kb * 128:(kb + 1) * 128],
                            rhs=xt[:, kt, :],
                            start=(kt == 0), stop=(kt == NKT - 1))
                    nc.vector.tensor_scalar_add(
                        dst[:, kb, t * 512:(t + 1) * 512], ps,
                        bias[:, kb:kb + 1])

            def proj_v(t):
                xt = xpool.tile([128, NKT, 512], BF16, tag="xt",
                                name=f"xt_v_{t}")
                nc.sync.dma_start(
                    out=xt,
                    in_=xvT.rearrange("(kt p) s -> p kt s", p=128)[:, :, t * 512:(t + 1) * 512])
                for m in range(4):
                    ps = pp.tile([128, 512], F32, tag="pp", name=f"ps_v_{t}_{m}")
                    for kt in range(NKT):
                        nc.tensor.matmul(
                            ps,
                            lhsT=xt[:, kt, m * 128:(m + 1) * 128],
                            rhs=wv_sb[:, kt, :],
                            start=(kt == 0), stop=(kt == NKT - 1))
                    sk = t * 4 + m
                    vdst = vb_sb[:, sk, :].rearrange(
                        "p (h dd) -> p h dd", dd=DD)[:, :, 0:HD]
                    nc.vector.tensor_add(
                        vdst,
                        ps.rearrange("p (h d) -> p h d", d=HD),
                        bvb_sb.rearrange("p (h d) -> p h d", d=HD))

            exp_state = [0]

            def attn(c):
                BLK = 4
                NB = NSK // BLK
                for kb in range(NKB):
                    zps = [zqpool.tile([DD, 512], F32, tag=f"z{d}",
                                       name=f"zps{d}_{kb}_{c}")
                           for d in range(2)]
                    pts = {}
                    for blk in range(NB + 1):
                        if blk < NB:
                            # scores burst: 64x128 row-pair mode
                            for sk in range(blk * BLK, (blk + 1) * BLK):
                                spt = sppool.tile([128, 2, 512], F32, tag="sp")
                                for d in range(2):
                                    nc.tensor.matmul(
                                        spt[:, d, :],
                                        lhsT=kt_sb[d * 64:(d + 1) * 64, kb,
                                                   sk * 128:(sk + 1) * 128],
                                        rhs=qt_sb[d * 64:(d + 1) * 64, kb,
                                                  c * 512:(c + 1) * 512],
                                        start=True, stop=True)
                                pt = ptpool.tile([128, 2, 512], BF16, tag="pt")
                                u = exp_state[0]
                                if EXP_DVE_EVERY and \
                                        u % EXP_DVE_EVERY == EXP_DVE_EVERY - 1:
                                    nc.vector.tensor_scalar(
                                        pt.bitcast(I16), spt,
                                        SCHRAUD_A, SCHRAUD_B,
                                        ALU.mult, ALU.add)
                                else:
                                    nc.scalar.activation(pt, spt, ACT.Exp,
                                                         scale=0.125)
                                exp_state[0] = u + 1
                                pts[sk] = pt
                        if blk >= 1:
                            # PV burst for previous block: 65-wide serial
                            for j in range((blk - 1) * BLK, blk * BLK):
                                pt = pts.pop(j)
                                for d in range(2):
                                    hh = 2 * kb + d
                                    nc.tensor.matmul(
                                        zps[d],
                                        lhsT=vb_sb[:, j, hh * DD:(hh + 1) * DD],
                                        rhs=pt[:, d, :],
                                        start=(j == 0), stop=(j == NSK - 1))

                    # normalize: z / denominator-row -> zt bf16
                    for d in range(2):
                        dcp = npool.tile([1, 512], F32, tag="dcp",
                                         name=f"dcp_{c}_{kb}_{d}")
                        nc.vector.tensor_copy(dcp, zps[d][HD:HD + 1, :])
                        rc = npool.tile([1, 512], F32, tag="rc",
                                        name=f"rc_{c}_{kb}_{d}")
                        nc.vector.reciprocal_approx_fast(rc, dcp)
                        bc = npool.tile([HD, 512], F32, tag="bc",
                                        name=f"bc_{c}_{kb}_{d}")
                        nc.gpsimd.partition_broadcast(bc, rc, channels=HD)
                        nc.vector.tensor_mul(
                            zt_sb[d * 64:(d + 1) * 64, kb, c * 512:(c + 1) * 512],
                            zps[d][0:HD, :], bc)

            def outproj(c):
                for t in range(4 * c, 4 * c + 4):
                    os_t = opool.tile([128, D], F32, tag="os", name=f"os_{t}")
                    for n in range(D // 512):
                        po = pp.tile([128, 512], F32, tag="pp",
                                     name=f"po_{t}_{n}")
                        for hb in range(NKB):
                            nc.tensor.matmul(
                                po,
                                lhsT=zt_sb[:, hb, t * 128:(t + 1) * 128],
                                rhs=wo_sb[:, hb, n * 512:(n + 1) * 512],
                                start=(hb == 0), stop=(hb == NKB - 1))
                        nc.vector.tensor_copy(os_t[:, n * 512:(n + 1) * 512], po)
                    nc.sync.dma_start(out=out[t * 128:(t + 1) * 128, :], in_=os_t)

            # ---------------- emission (= scheduler priority) ----------------
            for t in range(NT):
                proj_qk(xkT, wk_sb, kt_sb, bk_sb, t)
            proj_qk(xqT, wq_sb, qt_sb, bq_sb, 0)
            for t in range(NT):
                proj_v(t)
            for c in range(NT):
                attn(c)
                if c + 1 < NT:
                    proj_qk(xqT, wq_sb, qt_sb, bq_sb, c + 1)
                outproj(c)

        if rep_loop is not None:
            rep_loop.__exit__(None, None, None)

    nc.compile()
    return nc


_NC_CACHE = {}


def _get_nc(S=S_FULL, reps=1):
    key = (S, reps)
    if key not in _NC_CACHE:
        _NC_CACHE[key] = build_nc(S, reps=reps)
    return _NC_CACHE[key]


def make_in_maps(query, key, value, Wq, bq, Wk, bk, Wv, bv, Wo, bo):
    """Shard full inputs into 8 per-core input dicts (bf16 operands)."""
    bf = lambda a: np.ascontiguousarray(np.asarray(a, dtype=np.float32)).astype(ml_dtypes.bfloat16)
    f32 = lambda a: np.ascontiguousarray(np.asarray(a, dtype=np.float32))
    in_maps = []
    for core in range(N_CORES):
        b, hg = core // 2, core % 2
        sl = slice(hg * HH, (hg + 1) * HH)
        in_maps.append({
            "xqT": bf(np.asarray(query)[b].T),
            "xkT": bf(np.asarray(key)[b].T),
            "xvT": bf(np.asarray(value)[b].T),
            "wq": bf(np.asarray(Wq)[:, sl]),
            "wk": bf(np.asarray(Wk)[:, sl]),
            "wv": bf(np.asarray(Wv)[:, sl]),
            "wo": bf(np.asarray(Wo)[sl, :]),
            "bq": f32(np.asarray(bq)[sl]),
            "bk": f32(np.asarray(bk)[sl]),
            "bv": f32(np.asarray(bv)[sl]),
        })
    return in_maps


def kernel(query, key, value, Wq, bq, Wk, bk, Wv, bv, Wo, bo, **run_kwargs):
    nc = _get_nc(S_FULL)
    in_maps = make_in_maps(query, key, value, Wq, bq, Wk, bk, Wv, bv, Wo, bo)
    res = run_bass_kernel_spmd(nc, in_maps, core_ids=list(range(N_CORES)),
                               **run_kwargs)
    bo_np = np.asarray(bo, dtype=np.float32)
    outs = [np.asarray(r["out"], dtype=np.float32) for r in res.results]
    full = np.stack([outs[2 * b] + outs[2 * b + 1] + bo_np for b in range(B)])
    return full.astype(np.float32)
